# revision 18
# baseline (speedup 1.0000x reference)
"""Trainium2 Bass kernel for nn_DecisionMaking (dense_mlp, memory-bound).

Math (per batch b):
    h  = relu(op_emb@Wo [+bcast] + machine_emb@Wm [+bcast] + edge_emb@We + b1)
    h2 = relu(h @ W2 + b2)
    s  = (h2 @ W3)[...,0]           (+b3 cancels in log_softmax)
    out = log_softmax(where(mask, s, -inf)) over flattened N*M

Sharding: B=8 batches -> 8 NeuronCores, one full batch per core.  No
collectives: the log_softmax reduction is per-batch and stays on-core.

Per-core dataflow (N*M = 32768 rows of D=128, processed in 32 groups of
1024 rows):
  - DMA edge group -> SBUF nat [128, 1024]  (partition = row%128)
  - PE transpose 8x [128,128] -> PSUM, copy -> SBUF edgeT [128, 1024]
  - mm1: psum1[0:64]  = We^T @ edgeT[:, :512]   (tile_position (0,0))
         psum1[64:128]= We^T @ edgeT[:, 512:]   (tile_position (0,64))
  - DVE: tmp = psum1 + aug  (aug = precomputed op+machine+b1 outer sum)
  - GPSIMD: h1 = relu(tmp)
  - mm2: row+col packed pair (K=64 each half) -> psum2 [128, 512]
  - ACT: h2 = relu(psum2 + b2)
  - mm3: 4x (lhsT = h2[:, 128s:128s+128], rhs = W3-stacked [128,2])
         accumulating scores into a persistent PSUM bank, partition-dense
  - tail: transpose scores into row-order [64, 512], masked log_softmax,
          contiguous DMA out.
"""

import sys

for _p in ("/opt/trn_rl_repo",):
    if _p not in sys.path:
        sys.path.insert(0, _p)

from contextlib import ExitStack

import numpy as np

import concourse.bass as bass
import concourse.mybir as mybir
import concourse.tile as tile
from concourse import bacc
from concourse.masks import make_identity

F32 = mybir.dt.float32
F32R = mybir.dt.float32r
BF16 = mybir.dt.bfloat16
U8 = mybir.dt.uint8

B, N, M, D, H = 8, 512, 64, 128, 64
NM = N * M              # 32768 rows per batch
G = 32                  # row groups
GR = NM // G            # 1024 rows per group
NEG_BIG = -1.0e30

# --- perf knobs -------------------------------------------------------------
# matmul input dtype: F32 = exact (4 cyc/row), F32R = tf32 (1 cyc/row @N>=256)
MM_DT = F32
# dtype tag on the PE-transpose data path (pure permutation, values exact)
TR_DT = F32
# identity (moving operand) dtype must match the data path dtype width
# (walrus NCC_IBIR034 rejects mixed 32/16-bit matmul inputs)
TR_ID_DT = TR_DT


def _bc(ap, dt):
    """Bitcast an AP to a 4-byte sibling dtype (no data change)."""
    return ap.bitcast(dt) if dt != ap.dtype else ap


def build_module():
    nc = bacc.Bacc("TRN2", target_bir_lowering=False, debug=False)

    edge = nc.dram_tensor("edge", [NM, D], F32, kind="ExternalInput")
    opemb = nc.dram_tensor("opemb", [N, D], F32, kind="ExternalInput")
    mach = nc.dram_tensor("mach", [M, D], F32, kind="ExternalInput")
    mask = nc.dram_tensor("mask", [64, 512], U8, kind="ExternalInput")
    wo = nc.dram_tensor("wo", [D, H], F32, kind="ExternalInput")
    wm = nc.dram_tensor("wm", [D, H], F32, kind="ExternalInput")
    we = nc.dram_tensor("we", [D, H], F32, kind="ExternalInput")
    b1 = nc.dram_tensor("b1", [H, 1], F32, kind="ExternalInput")
    w2 = nc.dram_tensor("w2", [H, H], F32, kind="ExternalInput")
    b2 = nc.dram_tensor("b2", [H, 1], F32, kind="ExternalInput")
    w3 = nc.dram_tensor("w3", [H, 1], F32, kind="ExternalInput")
    out = nc.dram_tensor("out", [64, 512], F32, kind="ExternalOutput")

    AF = mybir.ActivationFunctionType

    with tile.TileContext(nc) as tc, ExitStack() as ctx:
        consts = ctx.enter_context(tc.tile_pool(name="consts", bufs=1))
        prep = ctx.enter_context(tc.tile_pool(name="prep", bufs=1))
        augp = ctx.enter_context(tc.tile_pool(name="augp", bufs=1))
        natp = ctx.enter_context(tc.tile_pool(name="natp", bufs=3))
        tedp = ctx.enter_context(tc.tile_pool(name="tedp", bufs=2))
        h1p = ctx.enter_context(tc.tile_pool(name="h1p", bufs=2))
        h2p = ctx.enter_context(tc.tile_pool(name="h2p", bufs=2))
        tmpp = ctx.enter_context(tc.tile_pool(name="tmpp", bufs=2))
        tailp = ctx.enter_context(tc.tile_pool(name="tailp", bufs=1))
        psA = ctx.enter_context(tc.tile_pool(name="psA", bufs=2, space="PSUM"))
        ps1 = ctx.enter_context(tc.tile_pool(name="ps1", bufs=2, space="PSUM"))
        ps2 = ctx.enter_context(tc.tile_pool(name="ps2", bufs=2, space="PSUM"))
        psS = ctx.enter_context(tc.tile_pool(name="psS", bufs=1, space="PSUM"))
        psP = ctx.enter_context(tc.tile_pool(name="psP", bufs=1, space="PSUM"))

        # ------------------------------------------------------------------
        # constants / weights
        #
        # walrus can encode only ONE semaphore wait on a self-loading fp32
        # Matmult (the LDWEIGHTS ISA struct).  Every tensor consumed by a PE
        # matmul is therefore produced by a DVE instruction (staging copies)
        # so consecutive matmuls see at most one unobserved semaphore; the
        # gpsimd-built identities are observed once via a dummy "absorber"
        # transpose that is the first PE instruction.
        # ------------------------------------------------------------------
        ident = consts.tile([128, 128], TR_ID_DT)
        make_identity(nc, ident)
        ident64 = consts.tile([64, 64], F32)
        make_identity(nc, ident64)

        wo_raw = consts.tile([D, H], F32, tag="wraw")
        nc.sync.dma_start(out=wo_raw, in_=wo[:, :])
        wm_raw = consts.tile([D, H], F32, tag="wraw2")
        nc.sync.dma_start(out=wm_raw, in_=wm[:, :])
        we_raw = consts.tile([D, H], F32, tag="wraw3")
        nc.sync.dma_start(out=we_raw, in_=we[:, :])
        w2raw = consts.tile([128, H], F32, tag="wraw4")
        nc.sync.dma_start(out=w2raw[0:64, :], in_=w2[:, :])
        nc.sync.dma_start(out=w2raw[64:128, :], in_=w2[:, :])
        w3raw = consts.tile([128, 2], F32, tag="wraw5")
        nc.vector.memset(w3raw, 0.0)
        nc.sync.dma_start(out=w3raw[0:64, 0:1], in_=w3[:, :])
        nc.sync.dma_start(out=w3raw[64:128, 1:2], in_=w3[:, :])

        # DVE staging copies (single producer engine for all PE operands)
        wo_sb = consts.tile([D, H], F32)
        nc.vector.tensor_copy(wo_sb, wo_raw)
        wm_sb = consts.tile([D, H], F32)
        nc.vector.tensor_copy(wm_sb, wm_raw)
        we_sb = consts.tile([D, H], F32)
        nc.vector.tensor_copy(we_sb, we_raw)
        w2st = consts.tile([128, H], F32)
        nc.vector.tensor_copy(w2st, w2raw)
        w3st = consts.tile([128, 2], F32)
        nc.vector.tensor_copy(w3st, w3raw)

        # b1 / b2 broadcast to 128 partitions ([0:64] == [64:128])
        b1bc = consts.tile([128, 1], F32)
        nc.sync.dma_start(out=b1bc, in_=bass.AP(b1, 0, [[0, 2], [1, 64], [0, 1]]))
        b2bc = consts.tile([128, 1], F32)
        nc.sync.dma_start(out=b2bc, in_=bass.AP(b2, 0, [[0, 2], [1, 64], [0, 1]]))

        mask_sb = consts.tile([64, 512], U8)
        nc.sync.dma_start(out=mask_sb, in_=mask[:, :])

        ones64 = consts.tile([64, 1], F32)
        nc.vector.memset(ones64, 1.0)
        negonesrow = consts.tile([1, 64], F32)
        nc.vector.memset(negonesrow, -1.0)

        # ------------------------------------------------------------------
        # prep: machine / op projections, aug = outer-sum bias table
        # ------------------------------------------------------------------
        mach_raw = prep.tile([M, D], F32, tag="mraw")
        nc.sync.dma_start(out=mach_raw, in_=mach[:, :])
        mach_nat = prep.tile([M, D], F32)
        nc.vector.tensor_copy(mach_nat, mach_raw)

        # absorber: first PE instruction; its single wait observes the
        # gpsimd identity builds so later transposes only wait on data
        psm = psP.tile([128, 64], F32, tag="psp")
        nc.tensor.transpose(psm[0:64, :], ident64, ident64)

        # machine_emb [64, 128] -> machT [128, 64]
        nc.tensor.transpose(psm, mach_nat, ident64)
        machT = prep.tile([D, M], F32)
        nc.vector.tensor_copy(machT, psm)

        # machb2[p, m] = (machine@Wm)[m, h=p%64] + b1[p%64], both halves equal
        psmb = psP.tile([128, 64], F32, tag="psp")
        nc.tensor.matmul(psmb[0:64, :], lhsT=wm_sb, rhs=machT,
                         start=True, stop=True, tile_position=(0, 0))
        nc.tensor.matmul(psmb[64:128, :], lhsT=wm_sb, rhs=machT,
                         start=True, stop=True, tile_position=(0, 64))
        machb2 = prep.tile([128, M], F32)
        nc.vector.tensor_scalar_add(machb2, psmb, b1bc[:, :])

        # op_emb [512, 128] -> op_embT [128, 512]
        op_raw = prep.tile([128, 512], F32, tag="opraw")
        nc.sync.dma_start(
            out=op_raw,
            in_=bass.AP(opemb, 0, [[128, 128], [16384, 4], [1, 128]]),
        )
        op_nat = prep.tile([128, 512], F32)
        nc.vector.tensor_copy(op_nat, op_raw)
        pso = psP.tile([128, 512], F32, tag="psp")
        for t in range(4):
            nc.tensor.matmul(
                _bc(pso[:, t * 128:(t + 1) * 128], TR_DT),
                lhsT=_bc(op_nat[:, t * 128:(t + 1) * 128], TR_DT),
                rhs=ident,
                start=True, stop=True, is_transpose=True,
            )
        op_embT = prep.tile([128, 512], F32)
        nc.vector.tensor_copy(op_embT, pso)

        # op_part2: rows 0-63 = (op@Wo)^T cols n=0..511,
        #           rows 64-127 = same but shifted by 8 ops (n=8..511)
        psq = psP.tile([128, 512], F32, tag="psp")
        nc.tensor.matmul(psq[0:64, 0:512], lhsT=wo_sb, rhs=op_embT,
                         start=True, stop=True, tile_position=(0, 0))
        nc.tensor.matmul(psq[64:128, 0:504], lhsT=wo_sb, rhs=op_embT[:, 8:512],
                         start=True, stop=True, tile_position=(0, 64))
        op_part2 = prep.tile([128, 512], F32)
        nc.vector.tensor_copy(op_part2[0:64, :], psq[0:64, :])
        nc.vector.tensor_copy(op_part2[64:128, 0:504], psq[64:128, 0:504])

        # aug[p, 512*g + 64*j + m] = op_part2[p, 16*g + j] + machb2[p, m]
        # built in 8 chunks of 4 groups so group 0 isn't gated on the whole
        aug = augp.tile([128, G * 512], F32)
        for c in range(8):
            src_op = bass.AP(
                op_part2.tensor,
                op_part2.offset + c * 64,
                [op_part2.ap[0], [16, 4], [1, 8], [0, 64]],
            )
            src_mb = bass.AP(
                machb2.tensor, machb2.offset,
                [machb2.ap[0], [0, 4], [0, 8], [1, 64]],
            )
            nc.gpsimd.tensor_tensor(
                aug[:, c * 2048:(c + 1) * 2048].rearrange(
                    "p (a b m) -> p a b m", a=4, b=8),
                src_op, src_mb, mybir.AluOpType.add)

        # ------------------------------------------------------------------
        # main loop over 32 groups of 1024 rows
        # ------------------------------------------------------------------
        tcat = psS.tile([128, 256], F32)  # persistent score accumulator

        for g in range(G):
            nat = natp.tile([128, GR], F32)
            nc.sync.dma_start(
                out=nat,
                in_=bass.AP(edge, g * GR * D, [[128, 128], [16384, 8], [1, 128]]),
            )

            ted = tedp.tile([128, GR], F32)
            for half in range(2):
                pst = psA.tile([128, 512], F32, tag="pst")
                for t in range(4):
                    tt = half * 4 + t
                    nc.tensor.matmul(
                        _bc(pst[:, t * 128:(t + 1) * 128], TR_DT),
                        lhsT=_bc(nat[:, tt * 128:(tt + 1) * 128], TR_DT),
                        rhs=ident,
                        start=True, stop=True, is_transpose=True,
                    )
                if half == 0:
                    nc.vector.tensor_copy(ted[:, 0:512], pst)
                else:
                    nc.scalar.copy(ted[:, 512:1024], pst)

            ps1t = ps1.tile([128, 512], F32)
            nc.tensor.matmul(ps1t[0:64, :], lhsT=_bc(we_sb, MM_DT),
                             rhs=_bc(ted[:, 0:512], MM_DT),
                             start=True, stop=True, tile_position=(0, 0))
            nc.tensor.matmul(ps1t[64:128, :], lhsT=_bc(we_sb, MM_DT),
                             rhs=_bc(ted[:, 512:1024], MM_DT),
                             start=True, stop=True, tile_position=(0, 64))

            tmp1 = tmpp.tile([128, 512], F32)
            nc.vector.tensor_add(tmp1, ps1t, aug[:, g * 512:(g + 1) * 512])
            h1 = h1p.tile([128, 512], F32)
            nc.gpsimd.tensor_relu(h1, tmp1)

            ps2t = ps2.tile([128, 512], F32)
            nc.tensor.matmul(ps2t[0:64, :], lhsT=_bc(w2st[0:64, :], MM_DT),
                             rhs=_bc(h1[0:64, :], MM_DT),
                             start=True, stop=True, tile_position=(0, 0))
            nc.tensor.matmul(ps2t[64:128, :], lhsT=_bc(w2st[64:128, :], MM_DT),
                             rhs=_bc(h1[64:128, :], MM_DT),
                             start=True, stop=True, tile_position=(64, 64))

            h2 = h2p.tile([128, 512], F32)
            nc.scalar.activation(h2, ps2t, AF.Relu, bias=b2bc[:, :])

            for s in range(4):
                nc.tensor.matmul(
                    tcat[:, 64 * s + 2 * g:64 * s + 2 * g + 2],
                    lhsT=_bc(h2[:, 128 * s:128 * (s + 1)], MM_DT),
                    rhs=_bc(w3st, MM_DT),
                    start=(g == 0 and s == 0), stop=(g == G - 1 and s == 3),
                )

        # ------------------------------------------------------------------
        # tail: reorder scores, masked log_softmax
        # ------------------------------------------------------------------
        # tcat[p, 64s+2g+u] = score(row 1024g + 512u + 128s + p)
        tcat_sb = tailp.tile([128, 256], F32)
        nc.vector.tensor_copy(tcat_sb, tcat)

        psr = psA.tile([64, 512], F32, tag="pst")
        for s in range(4):
            nc.tensor.matmul(
                _bc(psr[:, s * 128:(s + 1) * 128], TR_DT),
                lhsT=_bc(tcat_sb[:, s * 64:(s + 1) * 64], TR_DT),
                rhs=ident,
                start=True, stop=True, is_transpose=True,
            )
        # scores_sb[P, 128s+m] = score(512P + 128s + m)  -- row order
        scores_sb = tailp.tile([64, 512], F32)
        nc.vector.tensor_copy(scores_sb, psr)

        # finite masked copy for max/exp (masked -> -1e30)
        maskedf = tailp.tile([64, 512], F32)
        nc.vector.memset(maskedf, NEG_BIG)
        nc.vector.copy_predicated(maskedf, mask_sb, scores_sb)

        # global max
        mx = tailp.tile([64, 1], F32)
        nc.vector.tensor_reduce(mx, maskedf, mybir.AxisListType.X,
                                mybir.AluOpType.max)
        psmx = psP.tile([1, 64], F32, tag="psp")
        nc.tensor.transpose(psmx, mx, ident64)
        mxrow = tailp.tile([1, 64], F32)
        nc.vector.tensor_copy(mxrow, psmx)
        gmax = tailp.tile([1, 1], F32)
        nc.vector.tensor_reduce(gmax, mxrow, mybir.AxisListType.X,
                                mybir.AluOpType.max)

        # -gmax broadcast to [64, 1]
        psnb = psP.tile([64, 1], F32, tag="psp")
        nc.tensor.matmul(psnb, lhsT=negonesrow, rhs=gmax,
                         start=True, stop=True)
        ngmax = tailp.tile([64, 1], F32)
        nc.vector.tensor_copy(ngmax, psnb)

        # exp(x - gmax), with per-partition sums
        esb = tailp.tile([64, 512], F32)
        sums = tailp.tile([64, 1], F32)
        nc.scalar.activation(esb, maskedf, AF.Exp, bias=ngmax[:, :],
                             accum_out=sums)

        # Z = sum over partitions; shift = gmax + ln(Z)
        sums_d = tailp.tile([64, 1], F32)
        nc.vector.tensor_copy(sums_d, sums)
        psz = psP.tile([1, 1], F32, tag="psp")
        nc.tensor.matmul(psz, lhsT=ones64, rhs=sums_d, start=True, stop=True)
        zsb = tailp.tile([1, 1], F32)
        nc.vector.tensor_copy(zsb, psz)
        lz = tailp.tile([1, 1], F32)
        nc.scalar.activation(lz, zsb, AF.Ln)
        shift = tailp.tile([1, 1], F32)
        nc.vector.tensor_add(shift, lz, gmax)
        psns = psP.tile([64, 1], F32, tag="psp")
        nc.tensor.matmul(psns, lhsT=negonesrow, rhs=shift,
                         start=True, stop=True)
        nshift = tailp.tile([64, 1], F32)
        nc.vector.tensor_copy(nshift, psns)

        # out = where(mask, scores - shift, -inf)
        outf = tailp.tile([64, 512], F32)
        nc.vector.memset(outf, float("-inf"))
        shifted = tailp.tile([64, 512], F32)
        nc.vector.tensor_scalar_add(shifted, scores_sb, nshift[:, :])
        nc.vector.copy_predicated(outf, mask_sb, shifted)

        nc.sync.dma_start(out=out[:, :], in_=outf)

    nc.compile()
    return nc


_CACHED = None


def _get_module():
    global _CACHED
    if _CACHED is None:
        _CACHED = build_module()
    return _CACHED


def make_in_maps(inputs):
    op_emb = np.ascontiguousarray(inputs["op_emb"], dtype=np.float32)
    machine_emb = np.ascontiguousarray(inputs["machine_emb"], dtype=np.float32)
    edge_emb = np.ascontiguousarray(inputs["edge_emb"], dtype=np.float32)
    action_mask = np.asarray(inputs["action_mask"])
    W1 = np.ascontiguousarray(inputs["W1"], dtype=np.float32)
    b1 = np.ascontiguousarray(inputs["b1"], dtype=np.float32).reshape(H, 1)
    W2 = np.ascontiguousarray(inputs["W2"], dtype=np.float32)
    b2 = np.ascontiguousarray(inputs["b2"], dtype=np.float32).reshape(H, 1)
    W3 = np.ascontiguousarray(inputs["W3"], dtype=np.float32)

    wo, wm, we = W1[:D], W1[D:2 * D], W1[2 * D:]
    in_maps = []
    for b in range(B):
        in_maps.append({
            "edge": np.ascontiguousarray(edge_emb[b].reshape(NM, D)),
            "opemb": op_emb[b],
            "mach": machine_emb[b],
            "mask": np.ascontiguousarray(
                action_mask[b].reshape(64, 512).astype(np.uint8)),
            "wo": np.ascontiguousarray(wo),
            "wm": np.ascontiguousarray(wm),
            "we": np.ascontiguousarray(we),
            "b1": b1, "w2": W2, "b2": b2,
            "w3": np.ascontiguousarray(W3.reshape(H, 1)),
        })
    return in_maps


def kernel(**inputs) -> np.ndarray:
    from concourse.bass_utils import run_bass_kernel_spmd

    nc = _get_module()
    in_maps = make_in_maps(inputs)
    res = run_bass_kernel_spmd(nc, in_maps, core_ids=list(range(B)))
    out = np.stack([res.results[b]["out"].reshape(NM) for b in range(B)])
    return out.astype(np.float32)


if __name__ == "__main__":
    nc = build_module()
    print("module built ok; instructions:",
          sum(len(bb.instructions) for f in nc.m.functions for bb in f.blocks))
